# revision 13
# baseline (speedup 1.0000x reference)
"""Causal self-attention (B=2, T=2048, C=768, H=12) on 8 TRN2 NeuronCores.

Sharding: core (b, g) = batch b in {0,1} x head-group g in {0..3}; each core
owns 3 heads (192 of 768 channel dims). qkv + scores + softmax + att@v + its
slice of the output projection run fully on-device; the host sums the 4
row-parallel partial projections per batch.

Device dataflow (matmuls in WDT = float32r by default):
  B1: qkT [384, 2048] = wqkT.T @ xT      (q/k per head, head-dim on partitions)
  B2: v   [2048, 192] = xT.T @ wvT2      (natural layout, +ones col per head)
  C:  per (head, 512-wide q-chunk): scoresT [k,q] pairs of tiles in one PSUM
      2-bank tile -> additive causal mask bias (-1e9) on diagonal tiles ->
      exp (ACT, scale=1/8) -> av matmul with a ones-row appended to v so the
      softmax denominator accumulates in PSUM row 64 for free
  D:  reciprocal of denom row (fp32), PE-broadcast (K=1 ones matmul, fp32),
      scale yT into WDT tiles
  E:  out_partial [2048, 768] = yT.T @ wpT, DMA to DRAM
"""

import os
import numpy as np

T = 2048
C = 768
D = 64
G = 4            # head groups (cores per batch)
HPG = 3          # heads per group
NCORES = 8
QCHUNK = 512
KTILE = 128
NEG = -1.0e9     # additive causal mask bias

WDT_NAME = os.environ.get("KERNEL_WDT", "f32r")   # f32r | bf16 | f32

_compiled = None          # Bacc memo
LAST_RESULTS = None       # BassKernelResults of last run (for test.py)


def _wdt(mybir):
    return {"f32r": mybir.dt.float32r,
            "bf16": mybir.dt.bfloat16,
            "f32": mybir.dt.float32}[WDT_NAME]


def _np_wdt():
    if WDT_NAME == "bf16":
        import ml_dtypes
        return ml_dtypes.bfloat16
    return np.float32


def _build_bass():
    import concourse.bacc as bacc
    import concourse.mybir as mybir
    import concourse.tile as tile

    f32 = mybir.dt.float32
    wdt = _wdt(mybir)
    AF = mybir.ActivationFunctionType

    nc = bacc.Bacc("TRN2", target_bir_lowering=False, debug=False,
                   enable_asserts=False)

    xT_d = nc.dram_tensor("xT", [C, T], wdt, kind="ExternalInput")
    wqkT_d = nc.dram_tensor("wqkT", [C, 384], wdt, kind="ExternalInput")
    wvT2_d = nc.dram_tensor("wvT2", [C, 384], wdt, kind="ExternalInput")
    wp_d = [nc.dram_tensor(f"wp{h}", [D, C], wdt, kind="ExternalInput")
            for h in range(HPG)]
    maskb_d = nc.dram_tensor("maskb", [128, 4 * QCHUNK], f32,
                             kind="ExternalInput")
    out_d = nc.dram_tensor("out", [T, C], f32, kind="ExternalOutput")
    debug = bool(int(os.environ.get("KERNEL_DEBUG", "0")))
    if debug:
        dbg = {
            "d_qpair": nc.dram_tensor("d_qpair", [128, T], f32, kind="ExternalOutput"),
            "d_kpair": nc.dram_tensor("d_kpair", [128, T], f32, kind="ExternalOutput"),
            "d_qk2": nc.dram_tensor("d_qk2", [128, T], f32, kind="ExternalOutput"),
            "d_k2b": nc.dram_tensor("d_k2b", [64, T], f32, kind="ExternalOutput"),
            "d_vb0": nc.dram_tensor("d_vb0", [128, 195], f32, kind="ExternalOutput"),
            "d_yacc0": nc.dram_tensor("d_yacc0", [65, T], f32, kind="ExternalOutput"),
            "d_ytm0": nc.dram_tensor("d_ytm0", [64, T], f32, kind="ExternalOutput"),
            "d_e00": nc.dram_tensor("d_e00", [128, 1024], f32, kind="ExternalOutput"),
            "d_s00": nc.dram_tensor("d_s00", [128, 1024], f32, kind="ExternalOutput"),
        }

    with tile.TileContext(nc) as tc:
        with (
            tc.tile_pool(name="xt", bufs=1) as xt_pool,
            tc.tile_pool(name="qk", bufs=1) as qk_pool,
            tc.tile_pool(name="vb", bufs=1) as vb_pool,
            tc.tile_pool(name="wq", bufs=1) as wq_pool,
            tc.tile_pool(name="wv", bufs=1) as wv_pool,
            tc.tile_pool(name="wp", bufs=1) as wp_pool,
            tc.tile_pool(name="msk", bufs=1) as msk_pool,
            tc.tile_pool(name="yt", bufs=1) as yt_pool,
            tc.tile_pool(name="ex", bufs=3) as ex_pool,
            tc.tile_pool(name="ost", bufs=2) as ost_pool,
            tc.tile_pool(name="dbgp", bufs=1) as dbg_pool,
            tc.tile_pool(name="one", bufs=1) as one_pool,
            tc.tile_pool(name="psm", bufs=2, space="PSUM") as psm_pool,
            tc.tile_pool(name="psx", bufs=2, space="PSUM") as psx_pool,
        ):
            # ---- input DMAs ----
            xT = [xt_pool.tile([128, T], wdt, name=f"xT{k}") for k in range(6)]
            xT_view = xT_d.ap().rearrange("(k p) n -> k p n", p=128)
            for k in range(6):
                nc.sync.dma_start(out=xT[k][:], in_=xT_view[k])

            wqk = [wq_pool.tile([128, 384], wdt, name=f"wqk{k}") for k in range(6)]
            wqk_view = wqkT_d.ap().rearrange("(k p) n -> k p n", p=128)
            for k in range(6):
                nc.sync.dma_start(out=wqk[k][:], in_=wqk_view[k])

            wv2 = [wv_pool.tile([128, 384], wdt, name=f"wv2{k}") for k in range(6)]
            wv2_view = wvT2_d.ap().rearrange("(k p) n -> k p n", p=128)
            for k in range(6):
                nc.sync.dma_start(out=wv2[k][:], in_=wv2_view[k])

            wp = [wp_pool.tile([D, C], wdt, name=f"wpt{h}") for h in range(HPG)]
            for h in range(HPG):
                nc.sync.dma_start(out=wp[h][:], in_=wp_d[h].ap())

            maskb = msk_pool.tile([128, 4 * QCHUNK], f32, name="maskb")
            nc.sync.dma_start(out=maskb[:], in_=maskb_d.ap())

            ones64 = one_pool.tile([65, D], f32, name="ones64")
            nc.any.memset(ones64[:], 1.0)
            ones3 = one_pool.tile([128, 3], f32, name="ones3")
            nc.any.memset(ones3[:], 1.0)

            # ---- B1: qkT [384, 2048] ----
            # wqkT col order: [q0|q1] [k0|k1] [q2|k2]
            qpair = qk_pool.tile([128, T], wdt, name="qpair")
            kpair = qk_pool.tile([128, T], wdt, name="kpair")
            qk2 = qk_pool.tile([128, T], wdt, name="qk2")
            k2b = qk_pool.tile([64, T], wdt, name="k2b")
            b1_dst = [qpair, kpair, qk2]
            for m in range(3):
                for c in range(4):
                    ps = psm_pool.tile([128, 2 * QCHUNK], f32, name="psm",
                                       tag="psm")
                    for k in range(6):
                        nc.tensor.matmul(
                            ps[:, 0:QCHUNK],
                            wqk[k][:, m * 128:(m + 1) * 128],
                            xT[k][:, c * QCHUNK:(c + 1) * QCHUNK],
                            start=(k == 0), stop=(k == 5))
                    nc.any.tensor_copy(
                        out=b1_dst[m][:, c * QCHUNK:(c + 1) * QCHUNK],
                        in_=ps[:, 0:QCHUNK])
            # k_h2 copy to base partition 0 (DMA can shift partitions)
            nc.sync.dma_start(out=k2b[:], in_=qk2[64:128, :])

            q_of = {0: qpair[0:64, :], 1: qpair[64:128, :], 2: qk2[0:64, :]}
            k_of = {0: kpair[0:64, :], 1: kpair[64:128, :], 2: k2b[0:64, :]}

            # ---- B2: v natural + ones cols, layout [128, 195] per tok tile ----
            vb = [vb_pool.tile([128, 3 * 65], wdt, name=f"vb{t}") for t in range(16)]
            for t in range(16):
                ps = psx_pool.tile([128, 768], f32, name="psx", tag="psx")
                for k in range(6):
                    nc.tensor.matmul(
                        ps[:, 0:384],
                        xT[k][:, t * 128:(t + 1) * 128],
                        wv2[k][:],
                        start=(k == 0), stop=(k == 5))
                dst = vb[t][:].rearrange("p (h x) -> p h x", x=65)
                nc.any.tensor_copy(
                    out=dst[:, :, 0:64],
                    in_=ps[:, 0:192].rearrange("p (h d) -> p h d", d=64))
                nc.any.tensor_copy(
                    out=dst[:, :, 64:65],
                    in_=ones3[:].rearrange("p (x y) -> p x y", y=1))

            if debug:
                dstg = dbg_pool.tile([128, T], f32, name="dstg", tag="dstg")
                for nm, tl in (("d_qpair", qpair), ("d_kpair", kpair),
                               ("d_qk2", qk2)):
                    nc.any.tensor_copy(out=dstg[:], in_=tl[:])
                    nc.sync.dma_start(out=dbg[nm].ap(), in_=dstg[:])
                dstg2 = dbg_pool.tile([64, T], f32, name="dstg2", tag="dstg")
                nc.any.tensor_copy(out=dstg2[:], in_=k2b[:])
                nc.sync.dma_start(out=dbg["d_k2b"].ap(), in_=dstg2[:])
                dstg3 = dbg_pool.tile([128, 195], f32, name="dstg3", tag="dstg")
                nc.any.tensor_copy(out=dstg3[:], in_=vb[0][:])
                nc.sync.dma_start(out=dbg["d_vb0"].ap(), in_=dstg3[:])

            # ---- C: attention per (head, q-chunk); D: division ----
            yt_acc = [yt_pool.tile([65, T], f32, name=f"yta{h}")
                      for h in range(HPG)]
            yt_mm = [yt_pool.tile([64, T], wdt, name=f"ytm{h}")
                     for h in range(HPG)]
            for h in range(HPG):
                qh, kh = q_of[h], k_of[h]
                for c in range(4):
                    qs = slice(c * QCHUNK, (c + 1) * QCHUNK)
                    nkt = 4 * c + 4          # k-tiles: 0 .. 4c+3 (always even)
                    yps = psx_pool.tile([65, QCHUNK], f32, name="psy", tag="psx")
                    for jp in range(0, nkt, 2):
                        ps = psm_pool.tile([128, 2 * QCHUNK], f32, name="pss",
                                           tag="psm")
                        for u in range(2):
                            j = jp + u
                            nc.tensor.matmul(
                                ps[:, u * QCHUNK:(u + 1) * QCHUNK],
                                kh[:, j * KTILE:(j + 1) * KTILE],
                                qh[:, qs],
                                start=True, stop=True)
                        o = jp - 4 * c
                        if o >= 0:
                            nc.vector.tensor_add(
                                out=ps[:], in0=ps[:],
                                in1=maskb[:, o * QCHUNK:(o + 2) * QCHUNK])
                        e = ex_pool.tile([128, 2 * QCHUNK], wdt, name="e",
                                         tag="e")
                        if debug and h == 0 and c == 0 and jp == 0:
                            dstg5 = dbg_pool.tile([128, 1024], f32,
                                                  name="dstg5", tag="dstg")
                            nc.any.tensor_copy(out=dstg5[:], in_=ps[:])
                            nc.sync.dma_start(out=dbg["d_s00"].ap(),
                                              in_=dstg5[:])
                        nc.scalar.activation(e[:], ps[:], AF.Exp, scale=0.125)
                        if debug and h == 0 and c == 0 and jp == 0:
                            dstg6 = dbg_pool.tile([128, 1024], f32,
                                                  name="dstg6", tag="dstg")
                            nc.any.tensor_copy(out=dstg6[:], in_=e[:])
                            nc.sync.dma_start(out=dbg["d_e00"].ap(),
                                              in_=dstg6[:])
                        for u in range(2):
                            j = jp + u
                            nc.tensor.matmul(
                                yps[:],
                                vb[j][:, h * 65:(h + 1) * 65],
                                e[:, u * QCHUNK:(u + 1) * QCHUNK],
                                start=(j == 0), stop=(j == nkt - 1),
                                skip_group_check=True)
                    nc.any.tensor_copy(out=yt_acc[h][:, qs], in_=yps[:])

            # ---- D: softmax division via y * exp(-ln(denom)) ----
            for h in range(HPG):
                nc.scalar.activation(yt_acc[h][64:65, :], yt_acc[h][64:65, :],
                                     AF.Ln)
            for h in range(HPG):
                for c in range(4):
                    qs = slice(c * QCHUNK, (c + 1) * QCHUNK)
                    psb = psm_pool.tile([64, QCHUNK], f32, name="psb",
                                        tag="psm")
                    nc.tensor.matmul(
                        psb[:],
                        ones64[64:65, :],
                        yt_acc[h][64:65, qs],
                        start=True, stop=True)
                    e2 = ex_pool.tile([64, QCHUNK], f32, name="e2", tag="e2")
                    nc.scalar.activation(e2[:], psb[:], AF.Exp, scale=-1.0)
                    nc.vector.tensor_mul(
                        out=yt_mm[h][:, qs], in0=yt_acc[h][0:64, qs],
                        in1=e2[:])

            if debug:
                nc.sync.dma_start(out=dbg["d_yacc0"].ap(), in_=yt_acc[0][:])
                dstg4 = dbg_pool.tile([64, T], f32, name="dstg4", tag="dstg")
                nc.any.tensor_copy(out=dstg4[:], in_=yt_mm[0][:])
                nc.sync.dma_start(out=dbg["d_ytm0"].ap(), in_=dstg4[:])

            # ---- E: projection, row-parallel partial ----
            for t in range(16):
                po = psx_pool.tile([128, C], f32, name="po", tag="psx")
                for n0, n1 in ((0, 512), (512, 768)):
                    for h in range(HPG):
                        nc.tensor.matmul(
                            po[:, n0:n1],
                            yt_mm[h][:, t * 128:(t + 1) * 128],
                            wp[h][:, n0:n1],
                            start=(h == 0), stop=(h == HPG - 1))
                so = ost_pool.tile([128, C], f32, name="so", tag="so")
                nc.any.tensor_copy(out=so[:], in_=po[:])
                nc.sync.dma_start(
                    out=out_d.ap()[t * 128:(t + 1) * 128, :], in_=so[:])

    nc.compile()
    return nc


def _shard_inputs(x, W_attn, W_proj):
    npw = _np_wdt()
    maskb = np.zeros((128, 4, QCHUNK), np.float32)
    rr = np.arange(128)[:, None]
    cc = np.arange(QCHUNK)[None, :]
    for o in range(4):
        maskb[:, o, :] = np.where(cc >= rr + 128 * o, 0.0, NEG)
    maskb = np.ascontiguousarray(maskb.reshape(128, 4 * QCHUNK))

    in_maps = []
    for b in range(x.shape[0]):
        xT = np.ascontiguousarray(x[b].T.astype(npw))
        for g in range(G):
            heads = [HPG * g + i for i in range(HPG)]
            Wq = np.concatenate([W_attn[h * D:(h + 1) * D] for h in heads], 0)
            Wk = np.concatenate([W_attn[C + h * D:C + (h + 1) * D] for h in heads], 0)
            Wv = np.concatenate([W_attn[2 * C + h * D:2 * C + (h + 1) * D] for h in heads], 0)
            wqkT = np.ascontiguousarray(np.concatenate(
                [Wq[0:64], Wq[64:128], Wk[0:64], Wk[64:128],
                 Wq[128:192], Wk[128:192]], 0).T.astype(npw))
            wvT2 = np.ascontiguousarray(
                np.concatenate([Wv, Wv], 0).T.astype(npw))
            m = {"xT": xT, "wqkT": wqkT, "wvT2": wvT2, "maskb": maskb}
            for i, h in enumerate(heads):
                m[f"wp{i}"] = np.ascontiguousarray(
                    W_proj[:, h * D:(h + 1) * D].T.astype(npw))
            in_maps.append(m)
    return in_maps


def kernel(x, W_attn, W_proj):
    global _compiled, LAST_RESULTS
    from concourse import bass_utils

    x = np.asarray(x)
    if _compiled is None:
        _compiled = _build_bass()
    nc = _compiled

    in_maps = _shard_inputs(np.asarray(x, np.float32),
                            np.asarray(W_attn, np.float32),
                            np.asarray(W_proj, np.float32))
    trace = bool(int(os.environ.get("KERNEL_TRACE", "0")))
    if trace:
        _ensure_ntff_hook()
    res = bass_utils.run_bass_kernel_spmd(
        nc, in_maps, core_ids=list(range(NCORES)), trace=trace)
    LAST_RESULTS = res

    B = x.shape[0]
    out = np.zeros((B, T, C), np.float32)
    for b in range(B):
        acc = res.results[b * G + 0]["out"].astype(np.float32)
        for g in range(1, G):
            acc = acc + res.results[b * G + g]["out"]
        out[b] = acc
    return out


def _ensure_ntff_hook():
    """The RL agent image lacks antenv.axon_hooks; recreate it so
    run_bass_kernel_spmd(trace=True) can capture NTFF profiles."""
    import sys
    import types
    import ctypes
    import contextlib
    try:
        from antenv.axon_hooks import get_axon_ntff_profile_hook  # noqa: F401
        return
    except ImportError:
        pass
    import antenv

    holder = {"hook": None}
    mod = types.ModuleType("antenv.axon_hooks")
    mod.set_axon_ntff_profile_hook = lambda h: holder.__setitem__("hook", h)
    mod.get_axon_ntff_profile_hook = lambda: holder["hook"]
    sys.modules["antenv.axon_hooks"] = mod
    antenv.axon_hooks = mod

    so_path = "/opt/axon/libaxon_pjrt.so"
    if not os.path.exists(so_path):
        return
    lib = ctypes.CDLL(so_path)
    if not hasattr(lib, "axon_start_nrt_profile"):
        return
    lib.axon_start_nrt_profile.argtypes = [ctypes.POINTER(ctypes.c_int64),
                                           ctypes.c_size_t]
    lib.axon_start_nrt_profile.restype = ctypes.c_int64
    lib.axon_stop_nrt_profile.argtypes = [ctypes.c_char_p]
    lib.axon_stop_nrt_profile.restype = ctypes.c_int64

    @contextlib.contextmanager
    def _hook(output_dir, device_ids):
        import jax
        jax.devices()
        if device_ids:
            ids = (ctypes.c_int64 * len(device_ids))(*device_ids)
            rc = lib.axon_start_nrt_profile(ids, len(device_ids))
        else:
            rc = lib.axon_start_nrt_profile(None, 0)
        if rc != 0:
            raise RuntimeError(f"axon_start_nrt_profile rc={rc}")
        try:
            yield
        finally:
            n = lib.axon_stop_nrt_profile(str(output_dir).encode())
            print(f"ntff profile: {n} file(s) written to {output_dir}")

    mod.set_axon_ntff_profile_hook(_hook)
